# revision 34
# baseline (speedup 1.0000x reference)
"""Trainium2 Bass kernel for nn_BaselineSpanScorer (span-pair MLP scorer).

reference:
    xs        [32, 512, 1024] f32
    spans     [65536, 2] int   (begin/end token index within sequence)
    batch_ids [65536] int
    W1 [2048, 150], b1 [150], W2 [150, 17], b2 [17]
    out[n] = relu(concat(xs[b, s0], xs[b, s1]) @ W1 + b1) @ W2 + b2

Strategy (8 NeuronCores, data parallel with host routing):
  - Shard xs by batch: core c owns batches [4c, 4c+4) = 2048 token rows.
  - Algebraic factorization: per token t precompute
        A[t] = xs[t] @ W1[:1024],  G[t] = xs[t] @ W1[1024:]
    so pre[n] = A[i0_n] + G[i1_n] + b1.
  - Stage 1 writes a unified SBUF table: token t -> partition t%128,
    rank t//128, 1KB stripe [A(150 fp16) pad | G(150 fp16) pad].
  - Spans are bucketed by i0-block (16 blocks of 128 tokens) into a
    COMPILE-TIME slot template shared by all cores (per-block capacity =
    max over cores, short cores pad with dummy spans). A-side selection
    is then windowed one-hot matmuls whose column runs are compile-time:
    each span column is streamed exactly once.
  - G-side: SWDGE transpose dma_gather straight from the SBUF table.
    All descriptor generation is hoisted into stage 1 via prepare_only
    (descriptors depend only on indices); trigger_dma fires per queue
    once the table is complete.
  - Combine + bias + relu on DVE/ACT, [150]x[17] TensorE contraction,
    bias, DMA scores^T out.
  - Host scatters per-core outputs back to the original span order.

Compute dtype fp16 (rel err ~1e-3 vs f32 reference), f32 output.
"""

import os

os.environ.setdefault("MYCRO_LOCAL_CACHE", "1")

import numpy as np

# ---------------- problem constants (hardcoded per spec) ----------------
B, T, D = 32, 512, 1024
N_SPANS = 65536
H, L = 150, 17
NCORES = 8
BPC = B // NCORES        # batches per core = 4
TC = BPC * T             # tokens per core = 2048
N_KB = D // 128          # K blocks in stage 1 = 8
N_TT = TC // 128         # token tiles / table ranks = 16
SPAN_TILE = 512          # spans per stage-2 tile
W1N = 2 * H              # 300: stage-1 moving operand width (A | G)
RANK_ELEMS = 512         # fp16 elems per table rank stripe (1KB)
G_OFF = 256              # G features start at elem 256 of the stripe
PREP = os.environ.get("KERNEL_PREP", "1") == "1"   # hoist gather desc-gen
SBUF_TAB = os.environ.get("KERNEL_SBUF_TAB", "0") == "1"  # gather from SBUF


def build_graph(m_pad: int, runs):
    """Build the per-core SPMD Bass graph.

    m_pad: padded span count (multiple of SPAN_TILE); runs[st] is the
    compile-time list of (q, lo, hi) column runs for span tile st: slots
    [lo, hi) of the tile hold spans whose begin token lives in 128-token
    block q. Shared across cores by construction (slot template).
    """
    from concourse import bacc
    import concourse.mybir as mybir
    from concourse.tile import TileContext

    fp16 = mybir.dt.float16
    f32 = mybir.dt.float32
    i16 = mybir.dt.int16
    AF = mybir.ActivationFunctionType
    EQ = mybir.AluOpType.is_equal
    ADD = mybir.AluOpType.add

    n_st = m_pad // SPAN_TILE

    nc = bacc.Bacc(num_swdge_queues=4)

    xsi_d = nc.declare_dram_parameter("xsi", [128, N_TT * N_KB * 128], fp16, isOutput=False)
    wc_d = nc.declare_dram_parameter("wc", [128, N_KB * W1N], fp16, isOutput=False)
    w2p_d = nc.declare_dram_parameter("w2p", [128, 2 * L], fp16, isOutput=False)
    b1p_d = nc.declare_dram_parameter("b1p", [128, 2], f32, isOutput=False)
    b2p_d = nc.declare_dram_parameter("b2p", [L, 1], f32, isOutput=False)
    idx_d = nc.declare_dram_parameter("idx", [128, n_st * 32], i16, isOutput=False)
    i0v_d = nc.declare_dram_parameter("i0v", [128, m_pad], fp16, isOutput=False)
    iota_d = nc.declare_dram_parameter("iota", [128, SPAN_TILE], fp16, isOutput=False)
    outT_d = nc.declare_dram_parameter("outT", [L, m_pad], f32, isOutput=True)


    from concourse import library_config
    from concourse.tile_rust import add_dep_helper

    with TileContext(nc) as tc:
        with (
            tc.tile_pool(name="const", bufs=1) as constp,
            tc.tile_pool(name="xst", bufs=1) as xstp,
            tc.tile_pool(name="tab", bufs=1) as tabp,
            tc.tile_pool(name="dram", bufs=1, space="DRAM") as dramp,
            tc.tile_pool(name="gg", bufs=1) as ggp,
            tc.tile_pool(name="sr", bufs=1) as srp,
            tc.tile_pool(name="act", bufs=4) as actp,
            tc.tile_pool(name="ps1", bufs=2, space="PSUM") as ps1p,
            tc.tile_pool(name="pA0", bufs=2, space="PSUM") as pA0p,
            tc.tile_pool(name="pA1", bufs=2, space="PSUM") as pA1p,
            tc.tile_pool(name="ps2", bufs=2, space="PSUM") as ps2p,
            tc.tile_pool(name="ot", bufs=4) as otp,
        ):
            nc.gpsimd.load_library(library_config.mlp)

            # ---- input loads. scalar queue: small/metadata tensors the
            # preps and s_r builds need early; sync queue: xs chunks. ----
            idx_sb = constp.tile([128, n_st * 32], i16)
            nc.scalar.dma_start(out=idx_sb[:], in_=idx_d[:])
            wc_sb = constp.tile([128, N_KB * W1N], fp16)
            nc.scalar.dma_start(out=wc_sb[:], in_=wc_d[:])

            xst_sb = xstp.tile([128, N_TT, N_KB, 128], fp16)
            xsi_r = xsi_d.rearrange("p (tt kb j) -> p tt kb j", tt=N_TT, kb=N_KB)
            for ch in range(8):
                nc.sync.dma_start(
                    out=xst_sb[:, 2 * ch:2 * ch + 2, :, :],
                    in_=xsi_r[:, 2 * ch:2 * ch + 2, :, :],
                )

            iota_sb = constp.tile([128, SPAN_TILE], fp16)
            nc.scalar.dma_start(out=iota_sb[:], in_=iota_d[:])
            i0v_sb = constp.tile([128, m_pad], fp16)
            nc.scalar.dma_start(out=i0v_sb[:], in_=i0v_d[:])
            w2p_sb = constp.tile([128, 2 * L], fp16)
            nc.scalar.dma_start(out=w2p_sb[:], in_=w2p_d[:])
            b1p_sb = constp.tile([128, 2], f32)
            nc.scalar.dma_start(out=b1p_sb[:], in_=b1p_d[:])
            b2p_sb = constp.tile([L, 1], f32)
            nc.scalar.dma_start(out=b2p_sb[:], in_=b2p_d[:])

            # unified A|G table: [128 parts, rank, 2 halves, 256]
            tab = tabp.tile([128, N_TT, 2, G_OFF], fp16)
            # zero the junk tails so gathered payload is NaN-free
            nc.vector.memset(tab[:, :, 0, H:G_OFF], 0.0)
            nc.vector.memset(tab[:, :, 1, H:G_OFF], 0.0)

            tabg_t = None if SBUF_TAB else dramp.tile([TC, G_OFF], fp16)

            def emit_gather(out_ap, idx_ap, n_idx, queue, sem, prep):
                src = tab[:] if SBUF_TAB else tabg_t[:, :]
                kw = dict(
                    sbuf_tokens_per_rank=128,
                    sbuf_free_dim_per_rank=2 * G_OFF * 2,
                    sbuf_byte_offset=G_OFF * 2,
                ) if SBUF_TAB else {}
                return nc.gpsimd.dma_gather(
                    out_ap, src, idx_ap, n_idx, n_idx,
                    elem_size=G_OFF, transpose=True, prepare_only=prep,
                    sem=sem, queue_num=queue, **kw,
                )

            gg_tiles = []
            gsems = []
            dummy_sems = []
            if PREP:
                # Tile does not defer the gather's table dep to the trigger,
                # so emit the preps BEFORE the table is written: desc-gen
                # reads only idx; the data read happens at trigger time, and
                # the triggers below carry explicit table deps.
                for st in range(n_st):
                    gg = ggp.tile([128, 2, SPAN_TILE], fp16, tag=f"gg_{st}",
                                  name=f"gg_{st}")
                    gg_tiles.append(gg)
                    gsem = nc.alloc_semaphore(f"gsem{st}")
                    gsems.append(gsem)
                    emit_gather(gg[:], idx_sb[:, st * 32:(st + 1) * 32],
                                SPAN_TILE, st % 4, gsem, True)
                for q in range(4):
                    dscr = ggp.tile([128, 2, 128], fp16, tag=f"gg_dummy{q}",
                                    name=f"gg_dummy{q}")
                    dsem = nc.alloc_semaphore(f"gsemd{q}")
                    dummy_sems.append(dsem)
                    emit_gather(dscr[:], idx_sb[:, 0:8], 128, q, dsem, True)

            # ---- A-side one-hot tiles (DVE 2x; overlaps stage-1 PE work) ----
            s_r = []
            for st in range(n_st):
                s0 = srp.tile([128, SPAN_TILE], fp16, tag=f"s0_{st}",
                              name=f"s0_{st}")
                nc.vector.tensor_tensor(
                    out=s0[:],
                    in0=iota_sb[:],
                    in1=i0v_sb[:, st * SPAN_TILE:(st + 1) * SPAN_TILE],
                    op=EQ,
                )
                s_r.append(s0)

            # ---- stage 1: per token tile, 8 K-block matmuls + one
            # PSUM->SBUF copy into the table stripe ----
            tab_dma = None
            tab_dmas = []
            for tt in range(N_TT):
                ps = ps1p.tile([128, W1N], f32)
                for kb in range(N_KB):
                    nc.tensor.matmul(
                        ps[:],
                        xst_sb[:, tt, kb, :],
                        wc_sb[:, kb * W1N:(kb + 1) * W1N],
                        start=(kb == 0),
                        stop=(kb == N_KB - 1),
                    )
                psv = ps[:].rearrange("p (h x) -> p h x", h=2)
                nc.scalar.activation(tab[:, tt, :, 0:H], psv, AF.Copy)
                if not SBUF_TAB:
                    tab_dma = nc.sync.dma_start(
                        out=tabg_t[tt * 128:(tt + 1) * 128, :],
                        in_=tab[:, tt, 1, :],
                    )
                    tab_dmas.append(tab_dma)

            # ---- fire the prepared gathers (PREP) or emit direct gathers ----
            triggers = None
            if PREP:
                triggers = [
                    nc.gpsimd.trigger_dma(count=None, queue_num=q)
                    for q in range(4)
                ]
                # all 16 table writes (not mutually FIFO across queue rows):
                # pin every one on the first trigger; the rest follow it on
                # the GpSimd stream.
                deps = tab_dmas if not SBUF_TAB else []
                for d in deps:
                    add_dep_helper(triggers[0].ins, d.ins, True,
                                   "trigger after G table")
            else:
                for st in range(n_st):
                    gg = ggp.tile([128, 2, SPAN_TILE], fp16, tag=f"gg_{st}",
                                  name=f"gg_{st}")
                    gg_tiles.append(gg)
                    gsem = nc.alloc_semaphore(f"gsem{st}")
                    gsems.append(gsem)
                    g_inst = emit_gather(
                        gg[:], idx_sb[:, st * 32:(st + 1) * 32],
                        SPAN_TILE, st % 4, None, False,
                    ).then_inc(gsem, 16)
                    if not SBUF_TAB:
                        # first gather carries deps on ALL table writes (they
                        # are not mutually FIFO across queue rows); the rest
                        # follow on the GpSimd stream but keep the last-write
                        # dep for scheduler ordering.
                        deps = tab_dmas if st == 0 else [tab_dma]
                        for d in deps:
                            add_dep_helper(g_inst.ins, d.ins, True,
                                           "gather after G table")
                for q in range(4):
                    dscr = ggp.tile([128, 2, 128], fp16, tag=f"gg_dummy{q}",
                                    name=f"gg_dummy{q}")
                    dsem = nc.alloc_semaphore(f"gsemd{q}")
                    dummy_sems.append(dsem)
                    emit_gather(dscr[:], idx_sb[:, 0:8], 128, q, None,
                                False).then_inc(dsem, 16)

            # ---- stage 2 ----
            for st in range(n_st):
                gg = gg_tiles[st]
                s0 = s_r[st]
                pA0 = pA0p.tile([128, SPAN_TILE], f32, tag="pA0")
                pA1 = pA1p.tile([22, SPAN_TILE], f32, tag="pA1")
                for (q, lo, hi) in runs[st]:
                    nc.tensor.matmul(
                        pA0[:, lo:hi], tab[:, q, 0, 0:128], s0[:, lo:hi],
                        start=True, stop=True,
                    )
                    nc.tensor.matmul(
                        pA1[:, lo:hi], tab[:, q, 0, 128:H], s0[:, lo:hi],
                        start=True, stop=True,
                    )
                # pre-activation = A_onehot + b1 + G_gather, relu
                t0 = actp.tile([128, SPAN_TILE], fp16, tag="t0")
                t1 = actp.tile([22, SPAN_TILE], fp16, tag="t1")
                h0 = actp.tile([128, SPAN_TILE], fp16, tag="h0")
                h1 = actp.tile([22, SPAN_TILE], fp16, tag="h1")
                stt0 = nc.vector.scalar_tensor_tensor(
                    t0[:], pA0[:], b1p_sb[:, 0:1], gg[:, 0, :], ADD, ADD
                )
                stt1 = nc.vector.scalar_tensor_tensor(
                    t1[:], pA1[:], b1p_sb[0:22, 1:2], gg[0:22, 1, :], ADD, ADD
                )
                if PREP:
                    add_dep_helper(stt0.ins, triggers[st % 4].ins, True, "gg after trigger")
                    add_dep_helper(stt1.ins, triggers[st % 4].ins, True, "gg after trigger")
                # gg data only lands once the transfer completes AND the next
                # same-queue gather has pushed the xbar past it (drain
                # guarantee for the transpose-spray writes).
                stt0._wait_ge(gsems[st], 16)
                stt1._wait_ge(gsems[st], 16)
                nxt = st + 4
                guard = gsems[nxt] if nxt < n_st else dummy_sems[st % 4]
                gw = nc.vector.wait_ge(guard, 16)
                add_dep_helper(stt0.ins, gw.ins, True, "xbar drain guard")
                add_dep_helper(stt1.ins, gw.ins, True, "xbar drain guard")
                nc.scalar.activation(h0[:], t0[:], AF.Relu)
                nc.scalar.activation(h1[:], t1[:], AF.Relu)
                # W2 contraction
                ps2 = ps2p.tile([L, SPAN_TILE], f32, tag="ps2")
                nc.tensor.matmul(
                    ps2[:], w2p_sb[:, 0:L], h0[:], start=True, stop=False
                )
                nc.tensor.matmul(
                    ps2[:], w2p_sb[0:22, L:2 * L], h1[:], start=False, stop=True
                )
                ot = otp.tile([L, SPAN_TILE], f32)
                nc.scalar.activation(ot[:], ps2[:], AF.Identity, bias=b2p_sb[:])
                nc.sync.dma_start(
                    out=outT_d[:, st * SPAN_TILE:(st + 1) * SPAN_TILE], in_=ot[:]
                )

    return nc


def prep_inputs(xs, spans, batch_ids, W1, b1, W2, b2):
    """Host-side routing and layout.

    Returns (in_maps, scatter info, m_pad, runs)."""
    xs = np.asarray(xs, dtype=np.float32)
    spans = np.asarray(spans).astype(np.int64)
    batch_ids = np.asarray(batch_ids).astype(np.int64)
    W1 = np.asarray(W1, dtype=np.float32)
    b1 = np.asarray(b1, dtype=np.float32)
    W2 = np.asarray(W2, dtype=np.float32)
    b2 = np.asarray(b2, dtype=np.float32)

    core = batch_ids // BPC
    local0 = (batch_ids % BPC) * T + spans[:, 0]
    local1 = (batch_ids % BPC) * T + spans[:, 1]

    # bucket spans by (core, i0 block); slot template = per-block max
    q0 = local0 >> 7
    counts = np.zeros((NCORES, N_TT), np.int64)
    np.add.at(counts, (core, q0), 1)
    C = counts.max(axis=0)                      # [16] block capacities
    total = int(C.sum())
    m_pad = int(max(np.ceil(total / SPAN_TILE), 1) * SPAN_TILE)
    C[N_TT - 1] += m_pad - total                # absorb tail padding
    S = np.concatenate([[0], np.cumsum(C)])     # block slot offsets
    n_st = m_pad // SPAN_TILE

    # compile-time column runs per span tile
    runs = []
    for st in range(n_st):
        t_lo, t_hi = st * SPAN_TILE, (st + 1) * SPAN_TILE
        rl = []
        for q in range(N_TT):
            lo = max(t_lo, int(S[q]))
            hi = min(t_hi, int(S[q + 1]))
            if lo < hi:
                rl.append((q, lo - t_lo, hi - t_lo))
        runs.append(rl)

    # shared weights
    W1h = W1.astype(np.float16)
    wc = np.empty((128, N_KB * W1N), np.float16)
    for kb in range(N_KB):
        wc[:, kb * W1N:kb * W1N + H] = W1h[kb * 128:(kb + 1) * 128, :]
        wc[:, kb * W1N + H:(kb + 1) * W1N] = W1h[D + kb * 128:D + (kb + 1) * 128, :]
    W2pad = np.zeros((2 * 128, L), np.float16)
    W2pad[:H] = W2.astype(np.float16)
    w2p = np.empty((128, 2 * L), np.float16)
    w2p[:, 0:L] = W2pad[0:128]
    w2p[:, L:2 * L] = W2pad[128:256]
    b1pad = np.zeros((256,), np.float32)
    b1pad[:H] = b1
    b1p = np.ascontiguousarray(b1pad.reshape(2, 128).T)
    b2p = np.ascontiguousarray(b2.reshape(L, 1))

    iota = np.ascontiguousarray(
        np.broadcast_to(np.arange(128, dtype=np.float16)[:, None],
                        (128, SPAN_TILE))
    )

    # per-core slot assignment into the shared template
    scatter = []                                 # (orig span ids, slot ids)
    in_maps = []
    for c in range(NCORES):
        sel = np.nonzero(core == c)[0]
        qs = q0[sel]
        order = np.argsort(qs, kind="stable")
        sel = sel[order]
        qs = qs[order]
        # slots: for block q, spans land at S[q] + [0, n_cq)
        slot = np.empty(len(sel), np.int64)
        pos = 0
        i0 = np.zeros(m_pad, np.int64)
        i1 = np.zeros(m_pad, np.int64)
        # dummy defaults: block-consistent begin token, end token 0
        for q in range(N_TT):
            i0[int(S[q]):int(S[q + 1])] = q * 128
        for q in range(N_TT):
            n_cq = int(counts[c, q])
            slot[pos:pos + n_cq] = int(S[q]) + np.arange(n_cq)
            pos += n_cq
        i0[slot] = local0[sel]
        i1[slot] = local1[sel]
        scatter.append((sel, slot))

        # G gather indices: position i -> partition i%16, col i//16, 8x rep
        arr16 = i1.reshape(m_pad // 16, 16).T
        idxc = np.ascontiguousarray(np.tile(arr16, (8, 1)).astype(np.int16))
        # A-side one-hot compare values: r = i0 % 128, replicated
        i0r = (i0 & 127).astype(np.float16)
        i0v = np.ascontiguousarray(np.broadcast_to(i0r, (128, m_pad)))

        xs_c = xs[c * BPC:(c + 1) * BPC].reshape(TC, D)
        # interleave: [p, tt, kb, j] = xs_c[tt*128 + j, kb*128 + p]
        xsi = np.ascontiguousarray(
            xs_c.astype(np.float16)
            .reshape(N_TT, 128, N_KB, 128)
            .transpose(3, 0, 2, 1)
            .reshape(128, N_TT * N_KB * 128)
        )
        in_maps.append({
            "xsi": xsi, "wc": wc, "w2p": w2p, "b1p": b1p, "b2p": b2p,
            "idx": idxc, "i0v": i0v, "iota": iota,
        })

    return in_maps, scatter, m_pad, runs


def _scatter_out(results, scatter):
    out = np.empty((N_SPANS, L), np.float32)
    for c in range(NCORES):
        sel, slot = scatter[c]
        out[sel] = results[c]["outT"].T[slot]
    return out


def _install_ntff_shim():
    """Provide antenv.axon_hooks (missing on this image) so that
    run_bass_kernel_spmd(trace=True) can drive NTFF profiling via the
    axon .so. Only used by the profiling path."""
    import sys
    import types
    import ctypes
    import contextlib

    if "antenv.axon_hooks" in sys.modules:
        return
    import antenv

    holder = {"hook": None}
    mod = types.ModuleType("antenv.axon_hooks")
    mod.set_axon_ntff_profile_hook = lambda h: holder.__setitem__("hook", h)
    mod.get_axon_ntff_profile_hook = lambda: holder["hook"]
    sys.modules["antenv.axon_hooks"] = mod
    antenv.axon_hooks = mod

    so_path = "/opt/axon/libaxon_pjrt.so"
    try:
        lib = ctypes.CDLL(so_path)
    except OSError:
        return
    if not hasattr(lib, "axon_start_nrt_profile"):
        return
    lib.axon_start_nrt_profile.argtypes = [
        ctypes.POINTER(ctypes.c_int64),
        ctypes.c_size_t,
    ]
    lib.axon_start_nrt_profile.restype = ctypes.c_int64
    lib.axon_stop_nrt_profile.argtypes = [ctypes.c_char_p]
    lib.axon_stop_nrt_profile.restype = ctypes.c_int64

    @contextlib.contextmanager
    def _hook(output_dir, device_ids):
        import jax

        jax.devices()
        if device_ids:
            ids = (ctypes.c_int64 * len(device_ids))(*device_ids)
            rc = lib.axon_start_nrt_profile(ids, len(device_ids))
        else:
            rc = lib.axon_start_nrt_profile(None, 0)
        if rc != 0:
            raise RuntimeError(f"axon_start_nrt_profile rc={rc}")
        try:
            yield
        finally:
            n = lib.axon_stop_nrt_profile(str(output_dir).encode())
            print(f"profile: {n} file(s) written to {output_dir}")

    mod.set_axon_ntff_profile_hook(_hook)


def run(inputs: dict, trace: bool = False):
    """Run on the 8 NeuronCores. Returns (out, BassKernelResults)."""
    from concourse import bass_utils
    from concourse.bass_utils import run_bass_kernel_spmd

    if trace:
        _install_ntff_shim()
        bass_utils.upload_artifacts = lambda tmpdir: str(tmpdir)

    in_maps, scatter, m_pad, runs = prep_inputs(**inputs)
    nc = build_graph(m_pad, runs)
    nc.finalize()
    res = run_bass_kernel_spmd(
        nc, in_maps, list(range(NCORES)), trace=trace
    )
    return _scatter_out(res.results, scatter), res


def kernel(**inputs) -> np.ndarray:
    out, _ = run(inputs, trace=False)
    return out


# revision 37
# speedup vs baseline: 1.3175x; 1.3175x over previous
"""Trainium2 Bass kernel for nn_BaselineSpanScorer (span-pair MLP scorer).

reference:
    xs        [32, 512, 1024] f32
    spans     [65536, 2] int   (begin/end token index within sequence)
    batch_ids [65536] int
    W1 [2048, 150], b1 [150], W2 [150, 17], b2 [17]
    out[n] = relu(concat(xs[b, s0], xs[b, s1]) @ W1 + b1) @ W2 + b2

Strategy (8 NeuronCores, data parallel with host routing):
  - Shard xs by batch: core c owns batches [4c, 4c+4) = 2048 token rows.
  - Algebraic factorization: per token t precompute
        A[t] = xs[t] @ W1[:1024],  G[t] = xs[t] @ W1[1024:]
    so pre[n] = A[i0_n] + G[i1_n] + b1.
  - Stage 1 writes a unified SBUF table: token t -> partition t%128,
    rank t//128, 1KB stripe [A(150 fp16) pad | G(150 fp16) pad].
  - Spans are bucketed by i0-block (16 blocks of 128 tokens) into a
    COMPILE-TIME slot template shared by all cores (per-block capacity =
    max over cores, short cores pad with dummy spans). A-side selection
    is then windowed one-hot matmuls whose column runs are compile-time:
    each span column is streamed exactly once.
  - G-side: SWDGE transpose dma_gather straight from the SBUF table.
    All descriptor generation is hoisted into stage 1 via prepare_only
    (descriptors depend only on indices); trigger_dma fires per queue
    once the table is complete.
  - Combine + bias + relu on DVE/ACT, [150]x[17] TensorE contraction,
    bias, DMA scores^T out.
  - Host scatters per-core outputs back to the original span order.

Compute dtype fp16 (rel err ~1e-3 vs f32 reference), f32 output.
"""

import os

os.environ.setdefault("MYCRO_LOCAL_CACHE", "1")

import numpy as np

# ---------------- problem constants (hardcoded per spec) ----------------
B, T, D = 32, 512, 1024
N_SPANS = 65536
H, L = 150, 17
NCORES = 8
BPC = B // NCORES        # batches per core = 4
TC = BPC * T             # tokens per core = 2048
N_KB = D // 128          # K blocks in stage 1 = 8
N_TT = TC // 128         # token tiles / table ranks = 16
SPAN_TILE = 512          # spans per stage-2 tile
W1N = 2 * H              # 300: stage-1 moving operand width (A | G)
RANK_ELEMS = 512         # fp16 elems per table rank stripe (1KB)
G_OFF = 256              # G features start at elem 256 of the stripe
PREP = os.environ.get("KERNEL_PREP", "1") == "1"   # hoist gather desc-gen
SBUF_TAB = os.environ.get("KERNEL_SBUF_TAB", "0") == "1"  # gather from SBUF


def build_graph(m_pad: int, runs):
    """Build the per-core SPMD Bass graph.

    m_pad: padded span count (multiple of SPAN_TILE); runs[st] is the
    compile-time list of (q, lo, hi) column runs for span tile st: slots
    [lo, hi) of the tile hold spans whose begin token lives in 128-token
    block q. Shared across cores by construction (slot template).
    """
    from concourse import bacc
    import concourse.mybir as mybir
    from concourse.tile import TileContext

    fp16 = mybir.dt.float16
    f32 = mybir.dt.float32
    i16 = mybir.dt.int16
    AF = mybir.ActivationFunctionType
    EQ = mybir.AluOpType.is_equal
    ADD = mybir.AluOpType.add

    n_st = m_pad // SPAN_TILE

    nc = bacc.Bacc(num_swdge_queues=4)

    xsi_d = nc.declare_dram_parameter("xsi", [128, N_TT * N_KB * 128], fp16, isOutput=False)
    wc_d = nc.declare_dram_parameter("wc", [128, N_KB * W1N], fp16, isOutput=False)
    w2p_d = nc.declare_dram_parameter("w2p", [128, 2 * L], fp16, isOutput=False)
    b1p_d = nc.declare_dram_parameter("b1p", [128, 2], f32, isOutput=False)
    b2p_d = nc.declare_dram_parameter("b2p", [L, 1], f32, isOutput=False)
    i8 = mybir.dt.int8
    idx_d = nc.declare_dram_parameter("idx", [128, n_st * 32], i16, isOutput=False)
    i0v_d = nc.declare_dram_parameter("i0v", [128, m_pad], i8, isOutput=False)
    iota_d = nc.declare_dram_parameter("iota", [128, SPAN_TILE], i8, isOutput=False)
    outT_d = nc.declare_dram_parameter("outT", [L, m_pad], f32, isOutput=True)


    from concourse import library_config
    from concourse.tile_rust import add_dep_helper

    with TileContext(nc) as tc:
        with (
            tc.tile_pool(name="const", bufs=1) as constp,
            tc.tile_pool(name="xst", bufs=1) as xstp,
            tc.tile_pool(name="tab", bufs=1) as tabp,
            tc.tile_pool(name="dram", bufs=1, space="DRAM") as dramp,
            tc.tile_pool(name="gg", bufs=1) as ggp,
            tc.tile_pool(name="sr", bufs=1) as srp,
            tc.tile_pool(name="act", bufs=4) as actp,
            tc.tile_pool(name="ps1", bufs=2, space="PSUM") as ps1p,
            tc.tile_pool(name="pA0", bufs=2, space="PSUM") as pA0p,
            tc.tile_pool(name="pA1", bufs=2, space="PSUM") as pA1p,
            tc.tile_pool(name="ps2", bufs=2, space="PSUM") as ps2p,
            tc.tile_pool(name="ot", bufs=4) as otp,
        ):
            nc.gpsimd.load_library(library_config.mlp)

            # ---- input loads. scalar queue: small/metadata tensors the
            # preps and s_r builds need early; sync queue: xs chunks. ----
            idx_sb = constp.tile([128, n_st * 32], i16)
            nc.scalar.dma_start(out=idx_sb[:], in_=idx_d[:])
            wc_sb = constp.tile([128, N_KB * W1N], fp16)
            nc.scalar.dma_start(out=wc_sb[:], in_=wc_d[:])

            xst_sb = xstp.tile([128, N_TT, N_KB, 128], fp16)
            xsi_r = xsi_d.rearrange("p (tt kb j) -> p tt kb j", tt=N_TT, kb=N_KB)
            for ch in range(8):
                nc.sync.dma_start(
                    out=xst_sb[:, 2 * ch:2 * ch + 2, :, :],
                    in_=xsi_r[:, 2 * ch:2 * ch + 2, :, :],
                )

            iota_sb = constp.tile([128, SPAN_TILE], i8)
            nc.scalar.dma_start(out=iota_sb[:], in_=iota_d[:])
            i0v_sb = constp.tile([128, m_pad], i8)
            nc.scalar.dma_start(out=i0v_sb[:], in_=i0v_d[:])
            w2p_sb = constp.tile([128, 2 * L], fp16)
            nc.scalar.dma_start(out=w2p_sb[:], in_=w2p_d[:])
            b1p_sb = constp.tile([128, 2], f32)
            nc.scalar.dma_start(out=b1p_sb[:], in_=b1p_d[:])
            b2p_sb = constp.tile([L, 1], f32)
            nc.scalar.dma_start(out=b2p_sb[:], in_=b2p_d[:])

            # unified A|G table: [128 parts, rank, 2 halves, 256]
            tab = tabp.tile([128, N_TT, 2, G_OFF], fp16)
            # zero the junk tails so gathered payload is NaN-free
            nc.vector.memset(tab[:, :, 0, H:G_OFF], 0.0)
            nc.vector.memset(tab[:, :, 1, H:G_OFF], 0.0)

            tabg_t = None if SBUF_TAB else dramp.tile([TC, G_OFF], fp16)

            def emit_gather(out_ap, idx_ap, n_idx, queue, sem, prep):
                src = tab[:] if SBUF_TAB else tabg_t[:, :]
                kw = dict(
                    sbuf_tokens_per_rank=128,
                    sbuf_free_dim_per_rank=2 * G_OFF * 2,
                    sbuf_byte_offset=G_OFF * 2,
                ) if SBUF_TAB else {}
                return nc.gpsimd.dma_gather(
                    out_ap, src, idx_ap, n_idx, n_idx,
                    elem_size=G_OFF, transpose=True, prepare_only=prep,
                    sem=sem, queue_num=queue, **kw,
                )

            gg_tiles = []
            gsems = []
            dummy_sems = []
            if PREP:
                # Tile does not defer the gather's table dep to the trigger,
                # so emit the preps BEFORE the table is written: desc-gen
                # reads only idx; the data read happens at trigger time, and
                # the triggers below carry explicit table deps.
                for st in range(n_st):
                    gg = ggp.tile([128, 2, SPAN_TILE], fp16, tag=f"gg_{st}",
                                  name=f"gg_{st}")
                    gg_tiles.append(gg)
                    gsem = nc.alloc_semaphore(f"gsem{st}")
                    gsems.append(gsem)
                    emit_gather(gg[:], idx_sb[:, st * 32:(st + 1) * 32],
                                SPAN_TILE, st % 4, gsem, True)
                for q in range(4):
                    dscr = ggp.tile([128, 2, 128], fp16, tag=f"gg_dummy{q}",
                                    name=f"gg_dummy{q}")
                    dsem = nc.alloc_semaphore(f"gsemd{q}")
                    dummy_sems.append(dsem)
                    emit_gather(dscr[:], idx_sb[:, 0:8], 128, q, dsem, True)

            # ---- A-side one-hot tiles (DVE 2x; overlaps stage-1 PE work) ----
            s_r = []
            for st in range(n_st):
                s0 = srp.tile([128, SPAN_TILE], fp16, tag=f"s0_{st}",
                              name=f"s0_{st}")
                nc.vector.tensor_tensor(
                    out=s0[:],
                    in0=iota_sb[:],
                    in1=i0v_sb[:, st * SPAN_TILE:(st + 1) * SPAN_TILE],
                    op=EQ,
                )
                s_r.append(s0)

            # ---- stage 1: per token tile, 8 K-block matmuls + one
            # PSUM->SBUF copy into the table stripe ----
            tab_dma = None
            tab_dmas = []
            for tt in range(N_TT):
                ps = ps1p.tile([128, W1N], f32)
                for kb in range(N_KB):
                    nc.tensor.matmul(
                        ps[:],
                        xst_sb[:, tt, kb, :],
                        wc_sb[:, kb * W1N:(kb + 1) * W1N],
                        start=(kb == 0),
                        stop=(kb == N_KB - 1),
                    )
                psv = ps[:].rearrange("p (h x) -> p h x", h=2)
                nc.scalar.activation(tab[:, tt, :, 0:H], psv, AF.Copy)
                if not SBUF_TAB:
                    # scalar ring: don't queue behind the big xst loads
                    tab_dma = nc.scalar.dma_start(
                        out=tabg_t[tt * 128:(tt + 1) * 128, :],
                        in_=tab[:, tt, 1, :],
                    )
                    tab_dmas.append(tab_dma)

            # ---- fire the prepared gathers (PREP) or emit direct gathers ----
            triggers = None
            if PREP:
                triggers = [
                    nc.gpsimd.trigger_dma(count=None, queue_num=q)
                    for q in range(4)
                ]
                # all 16 table writes (not mutually FIFO across queue rows):
                # pin every one on the first trigger; the rest follow it on
                # the GpSimd stream.
                deps = tab_dmas if not SBUF_TAB else []
                for d in deps:
                    add_dep_helper(triggers[0].ins, d.ins, True,
                                   "trigger after G table")
            else:
                for st in range(n_st):
                    gg = ggp.tile([128, 2, SPAN_TILE], fp16, tag=f"gg_{st}",
                                  name=f"gg_{st}")
                    gg_tiles.append(gg)
                    gsem = nc.alloc_semaphore(f"gsem{st}")
                    gsems.append(gsem)
                    g_inst = emit_gather(
                        gg[:], idx_sb[:, st * 32:(st + 1) * 32],
                        SPAN_TILE, st % 4, None, False,
                    ).then_inc(gsem, 16)
                    if not SBUF_TAB:
                        # first gather carries deps on ALL table writes (they
                        # are not mutually FIFO across queue rows); the rest
                        # follow on the GpSimd stream but keep the last-write
                        # dep for scheduler ordering.
                        deps = tab_dmas if st == 0 else [tab_dma]
                        for d in deps:
                            add_dep_helper(g_inst.ins, d.ins, True,
                                           "gather after G table")
                for q in range(4):
                    dscr = ggp.tile([128, 2, 128], fp16, tag=f"gg_dummy{q}",
                                    name=f"gg_dummy{q}")
                    dsem = nc.alloc_semaphore(f"gsemd{q}")
                    dummy_sems.append(dsem)
                    emit_gather(dscr[:], idx_sb[:, 0:8], 128, q, None,
                                False).then_inc(dsem, 16)

            # ---- stage 2 ----
            for st in range(n_st):
                gg = gg_tiles[st]
                s0 = s_r[st]
                pA0 = pA0p.tile([128, SPAN_TILE], f32, tag="pA0")
                pA1 = pA1p.tile([22, SPAN_TILE], f32, tag="pA1")
                for (q, lo, hi) in runs[st]:
                    nc.tensor.matmul(
                        pA0[:, lo:hi], tab[:, q, 0, 0:128], s0[:, lo:hi],
                        start=True, stop=True,
                    )
                    nc.tensor.matmul(
                        pA1[:, lo:hi], tab[:, q, 0, 128:H], s0[:, lo:hi],
                        start=True, stop=True,
                    )
                # pre-activation = A_onehot + b1 + G_gather, relu
                t0 = actp.tile([128, SPAN_TILE], fp16, tag="t0")
                t1 = actp.tile([22, SPAN_TILE], fp16, tag="t1")
                h0 = actp.tile([128, SPAN_TILE], fp16, tag="h0")
                h1 = actp.tile([22, SPAN_TILE], fp16, tag="h1")
                stt0 = nc.vector.scalar_tensor_tensor(
                    t0[:], pA0[:], b1p_sb[:, 0:1], gg[:, 0, :], ADD, ADD
                )
                stt1 = nc.vector.scalar_tensor_tensor(
                    t1[:], pA1[:], b1p_sb[0:22, 1:2], gg[0:22, 1, :], ADD, ADD
                )
                if PREP:
                    add_dep_helper(stt0.ins, triggers[st % 4].ins, True, "gg after trigger")
                    add_dep_helper(stt1.ins, triggers[st % 4].ins, True, "gg after trigger")
                # gg data only lands once the transfer completes AND the next
                # same-queue gather has pushed the xbar past it (drain
                # guarantee for the transpose-spray writes).
                stt0._wait_ge(gsems[st], 16)
                stt1._wait_ge(gsems[st], 16)
                nxt = st + 4
                guard = gsems[nxt] if nxt < n_st else dummy_sems[st % 4]
                gw = nc.vector.wait_ge(guard, 16)
                add_dep_helper(stt0.ins, gw.ins, True, "xbar drain guard")
                add_dep_helper(stt1.ins, gw.ins, True, "xbar drain guard")
                nc.scalar.activation(h0[:], t0[:], AF.Relu)
                nc.scalar.activation(h1[:], t1[:], AF.Relu)
                # W2 contraction
                ps2 = ps2p.tile([L, SPAN_TILE], f32, tag="ps2")
                nc.tensor.matmul(
                    ps2[:], w2p_sb[:, 0:L], h0[:], start=True, stop=False
                )
                nc.tensor.matmul(
                    ps2[:], w2p_sb[0:22, L:2 * L], h1[:], start=False, stop=True
                )
                ot = otp.tile([L, SPAN_TILE], f32)
                nc.scalar.activation(ot[:], ps2[:], AF.Identity, bias=b2p_sb[:])
                nc.sync.dma_start(
                    out=outT_d[:, st * SPAN_TILE:(st + 1) * SPAN_TILE], in_=ot[:]
                )

    return nc


def prep_inputs(xs, spans, batch_ids, W1, b1, W2, b2):
    """Host-side routing and layout.

    Returns (in_maps, scatter info, m_pad, runs)."""
    xs = np.asarray(xs, dtype=np.float32)
    spans = np.asarray(spans).astype(np.int64)
    batch_ids = np.asarray(batch_ids).astype(np.int64)
    W1 = np.asarray(W1, dtype=np.float32)
    b1 = np.asarray(b1, dtype=np.float32)
    W2 = np.asarray(W2, dtype=np.float32)
    b2 = np.asarray(b2, dtype=np.float32)

    core = batch_ids // BPC
    local0 = (batch_ids % BPC) * T + spans[:, 0]
    local1 = (batch_ids % BPC) * T + spans[:, 1]

    # bucket spans by (core, i0 block); slot template = per-block max
    q0 = local0 >> 7
    counts = np.zeros((NCORES, N_TT), np.int64)
    np.add.at(counts, (core, q0), 1)
    C = counts.max(axis=0)                      # [16] block capacities
    total = int(C.sum())
    m_pad = int(max(np.ceil(total / SPAN_TILE), 1) * SPAN_TILE)
    C[N_TT - 1] += m_pad - total                # absorb tail padding
    S = np.concatenate([[0], np.cumsum(C)])     # block slot offsets
    n_st = m_pad // SPAN_TILE

    # compile-time column runs per span tile
    runs = []
    for st in range(n_st):
        t_lo, t_hi = st * SPAN_TILE, (st + 1) * SPAN_TILE
        rl = []
        for q in range(N_TT):
            lo = max(t_lo, int(S[q]))
            hi = min(t_hi, int(S[q + 1]))
            if lo < hi:
                rl.append((q, lo - t_lo, hi - t_lo))
        runs.append(rl)

    # shared weights
    W1h = W1.astype(np.float16)
    wc = np.empty((128, N_KB * W1N), np.float16)
    for kb in range(N_KB):
        wc[:, kb * W1N:kb * W1N + H] = W1h[kb * 128:(kb + 1) * 128, :]
        wc[:, kb * W1N + H:(kb + 1) * W1N] = W1h[D + kb * 128:D + (kb + 1) * 128, :]
    W2pad = np.zeros((2 * 128, L), np.float16)
    W2pad[:H] = W2.astype(np.float16)
    w2p = np.empty((128, 2 * L), np.float16)
    w2p[:, 0:L] = W2pad[0:128]
    w2p[:, L:2 * L] = W2pad[128:256]
    b1pad = np.zeros((256,), np.float32)
    b1pad[:H] = b1
    b1p = np.ascontiguousarray(b1pad.reshape(2, 128).T)
    b2p = np.ascontiguousarray(b2.reshape(L, 1))

    iota = np.ascontiguousarray(
        np.broadcast_to(np.arange(128, dtype=np.int8)[:, None],
                        (128, SPAN_TILE))
    )

    # per-core slot assignment into the shared template
    scatter = []                                 # (orig span ids, slot ids)
    in_maps = []
    for c in range(NCORES):
        sel = np.nonzero(core == c)[0]
        qs = q0[sel]
        order = np.argsort(qs, kind="stable")
        sel = sel[order]
        qs = qs[order]
        # slots: for block q, spans land at S[q] + [0, n_cq)
        slot = np.empty(len(sel), np.int64)
        pos = 0
        i0 = np.zeros(m_pad, np.int64)
        i1 = np.zeros(m_pad, np.int64)
        # dummy defaults: block-consistent begin token, end token 0
        for q in range(N_TT):
            i0[int(S[q]):int(S[q + 1])] = q * 128
        for q in range(N_TT):
            n_cq = int(counts[c, q])
            slot[pos:pos + n_cq] = int(S[q]) + np.arange(n_cq)
            pos += n_cq
        i0[slot] = local0[sel]
        i1[slot] = local1[sel]
        scatter.append((sel, slot))

        # G gather indices: position i -> partition i%16, col i//16, 8x rep
        arr16 = i1.reshape(m_pad // 16, 16).T
        idxc = np.ascontiguousarray(np.tile(arr16, (8, 1)).astype(np.int16))
        # A-side one-hot compare values: r = i0 % 128, replicated
        i0r = (i0 & 127).astype(np.int8)
        i0v = np.ascontiguousarray(np.broadcast_to(i0r, (128, m_pad)))

        xs_c = xs[c * BPC:(c + 1) * BPC].reshape(TC, D)
        # interleave: [p, tt, kb, j] = xs_c[tt*128 + j, kb*128 + p]
        xsi = np.ascontiguousarray(
            xs_c.astype(np.float16)
            .reshape(N_TT, 128, N_KB, 128)
            .transpose(3, 0, 2, 1)
            .reshape(128, N_TT * N_KB * 128)
        )
        in_maps.append({
            "xsi": xsi, "wc": wc, "w2p": w2p, "b1p": b1p, "b2p": b2p,
            "idx": idxc, "i0v": i0v, "iota": iota,
        })

    return in_maps, scatter, m_pad, runs


def _scatter_out(results, scatter):
    out = np.empty((N_SPANS, L), np.float32)
    for c in range(NCORES):
        sel, slot = scatter[c]
        out[sel] = results[c]["outT"].T[slot]
    return out


def _install_ntff_shim():
    """Provide antenv.axon_hooks (missing on this image) so that
    run_bass_kernel_spmd(trace=True) can drive NTFF profiling via the
    axon .so. Only used by the profiling path."""
    import sys
    import types
    import ctypes
    import contextlib

    if "antenv.axon_hooks" in sys.modules:
        return
    import antenv

    holder = {"hook": None}
    mod = types.ModuleType("antenv.axon_hooks")
    mod.set_axon_ntff_profile_hook = lambda h: holder.__setitem__("hook", h)
    mod.get_axon_ntff_profile_hook = lambda: holder["hook"]
    sys.modules["antenv.axon_hooks"] = mod
    antenv.axon_hooks = mod

    so_path = "/opt/axon/libaxon_pjrt.so"
    try:
        lib = ctypes.CDLL(so_path)
    except OSError:
        return
    if not hasattr(lib, "axon_start_nrt_profile"):
        return
    lib.axon_start_nrt_profile.argtypes = [
        ctypes.POINTER(ctypes.c_int64),
        ctypes.c_size_t,
    ]
    lib.axon_start_nrt_profile.restype = ctypes.c_int64
    lib.axon_stop_nrt_profile.argtypes = [ctypes.c_char_p]
    lib.axon_stop_nrt_profile.restype = ctypes.c_int64

    @contextlib.contextmanager
    def _hook(output_dir, device_ids):
        import jax

        jax.devices()
        if device_ids:
            ids = (ctypes.c_int64 * len(device_ids))(*device_ids)
            rc = lib.axon_start_nrt_profile(ids, len(device_ids))
        else:
            rc = lib.axon_start_nrt_profile(None, 0)
        if rc != 0:
            raise RuntimeError(f"axon_start_nrt_profile rc={rc}")
        try:
            yield
        finally:
            n = lib.axon_stop_nrt_profile(str(output_dir).encode())
            print(f"profile: {n} file(s) written to {output_dir}")

    mod.set_axon_ntff_profile_hook(_hook)


def run(inputs: dict, trace: bool = False):
    """Run on the 8 NeuronCores. Returns (out, BassKernelResults)."""
    from concourse import bass_utils
    from concourse.bass_utils import run_bass_kernel_spmd

    if trace:
        _install_ntff_shim()
        bass_utils.upload_artifacts = lambda tmpdir: str(tmpdir)

    in_maps, scatter, m_pad, runs = prep_inputs(**inputs)
    nc = build_graph(m_pad, runs)
    nc.finalize()
    res = run_bass_kernel_spmd(
        nc, in_maps, list(range(NCORES)), trace=trace
    )
    return _scatter_out(res.results, scatter), res


def kernel(**inputs) -> np.ndarray:
    out, _ = run(inputs, trace=False)
    return out


# revision 38
# speedup vs baseline: 1.3295x; 1.0091x over previous
"""Trainium2 Bass kernel for nn_BaselineSpanScorer (span-pair MLP scorer).

reference:
    xs        [32, 512, 1024] f32
    spans     [65536, 2] int   (begin/end token index within sequence)
    batch_ids [65536] int
    W1 [2048, 150], b1 [150], W2 [150, 17], b2 [17]
    out[n] = relu(concat(xs[b, s0], xs[b, s1]) @ W1 + b1) @ W2 + b2

Strategy (8 NeuronCores, data parallel with host routing):
  - Shard xs by batch: core c owns batches [4c, 4c+4) = 2048 token rows.
  - Algebraic factorization: per token t precompute
        A[t] = xs[t] @ W1[:1024],  G[t] = xs[t] @ W1[1024:]
    so pre[n] = A[i0_n] + G[i1_n] + b1.
  - Stage 1 writes a unified SBUF table: token t -> partition t%128,
    rank t//128, 1KB stripe [A(150 fp16) pad | G(150 fp16) pad].
  - Spans are bucketed by i0-block (16 blocks of 128 tokens) into a
    COMPILE-TIME slot template shared by all cores (per-block capacity =
    max over cores, short cores pad with dummy spans). A-side selection
    is then windowed one-hot matmuls whose column runs are compile-time:
    each span column is streamed exactly once.
  - G-side: SWDGE transpose dma_gather straight from the SBUF table.
    All descriptor generation is hoisted into stage 1 via prepare_only
    (descriptors depend only on indices); trigger_dma fires per queue
    once the table is complete.
  - Combine + bias + relu on DVE/ACT, [150]x[17] TensorE contraction,
    bias, DMA scores^T out.
  - Host scatters per-core outputs back to the original span order.

Compute dtype fp16 (rel err ~1e-3 vs f32 reference), f32 output.
"""

import os

os.environ.setdefault("MYCRO_LOCAL_CACHE", "1")

import numpy as np

# ---------------- problem constants (hardcoded per spec) ----------------
B, T, D = 32, 512, 1024
N_SPANS = 65536
H, L = 150, 17
NCORES = 8
BPC = B // NCORES        # batches per core = 4
TC = BPC * T             # tokens per core = 2048
N_KB = D // 128          # K blocks in stage 1 = 8
N_TT = TC // 128         # token tiles / table ranks = 16
SPAN_TILE = 512          # spans per stage-2 tile
W1N = 2 * H              # 300: stage-1 moving operand width (A | G)
RANK_ELEMS = 512         # fp16 elems per table rank stripe (1KB)
G_OFF = 256              # G features start at elem 256 of the stripe
PREP = os.environ.get("KERNEL_PREP", "1") == "1"   # hoist gather desc-gen
SBUF_TAB = os.environ.get("KERNEL_SBUF_TAB", "0") == "1"  # gather from SBUF


def build_graph(m_pad: int, runs):
    """Build the per-core SPMD Bass graph.

    m_pad: padded span count (multiple of SPAN_TILE); runs[st] is the
    compile-time list of (q, lo, hi) column runs for span tile st: slots
    [lo, hi) of the tile hold spans whose begin token lives in 128-token
    block q. Shared across cores by construction (slot template).
    """
    from concourse import bacc
    import concourse.mybir as mybir
    from concourse.tile import TileContext

    fp16 = mybir.dt.float16
    f32 = mybir.dt.float32
    i16 = mybir.dt.int16
    AF = mybir.ActivationFunctionType
    EQ = mybir.AluOpType.is_equal
    ADD = mybir.AluOpType.add

    n_st = m_pad // SPAN_TILE

    nc = bacc.Bacc(num_swdge_queues=4)

    xsi_d = nc.declare_dram_parameter("xsi", [128, N_TT * N_KB * 128], fp16, isOutput=False)
    wc_d = nc.declare_dram_parameter("wc", [128, N_KB * W1N], fp16, isOutput=False)
    w2p_d = nc.declare_dram_parameter("w2p", [128, 2 * L], fp16, isOutput=False)
    b1p_d = nc.declare_dram_parameter("b1p", [128, 2], f32, isOutput=False)
    b2p_d = nc.declare_dram_parameter("b2p", [L, 1], f32, isOutput=False)
    i8 = mybir.dt.int8
    idx_d = nc.declare_dram_parameter("idx", [128, n_st * 32], i16, isOutput=False)
    i0v_d = nc.declare_dram_parameter("i0v", [128, m_pad], i8, isOutput=False)
    iota_d = nc.declare_dram_parameter("iota", [128, SPAN_TILE], i8, isOutput=False)
    outT_d = nc.declare_dram_parameter("outT", [L, m_pad], f32, isOutput=True)


    from concourse import library_config
    from concourse.tile_rust import add_dep_helper

    with TileContext(nc) as tc:
        with (
            tc.tile_pool(name="const", bufs=1) as constp,
            tc.tile_pool(name="xst", bufs=1) as xstp,
            tc.tile_pool(name="tab", bufs=1) as tabp,
            tc.tile_pool(name="dram", bufs=1, space="DRAM") as dramp,
            tc.tile_pool(name="gg", bufs=1) as ggp,
            tc.tile_pool(name="sr", bufs=1) as srp,
            tc.tile_pool(name="act", bufs=4) as actp,
            tc.tile_pool(name="ps1", bufs=2, space="PSUM") as ps1p,
            tc.tile_pool(name="pA0", bufs=2, space="PSUM") as pA0p,
            tc.tile_pool(name="pA1", bufs=2, space="PSUM") as pA1p,
            tc.tile_pool(name="ps2", bufs=2, space="PSUM") as ps2p,
            tc.tile_pool(name="ot", bufs=4) as otp,
        ):
            nc.gpsimd.load_library(library_config.mlp)

            # ---- input loads. scalar queue: small/metadata tensors the
            # preps and s_r builds need early; sync queue: xs chunks. ----
            wc_sb = constp.tile([128, N_KB * W1N], fp16)
            nc.scalar.dma_start(out=wc_sb[:], in_=wc_d[:])
            idx_sb = constp.tile([128, n_st * 32], i16)
            nc.scalar.dma_start(out=idx_sb[:], in_=idx_d[:])

            xst_sb = xstp.tile([128, N_TT, N_KB, 128], fp16)
            xsi_r = xsi_d.rearrange("p (tt kb j) -> p tt kb j", tt=N_TT, kb=N_KB)
            for ch in range(8):
                nc.sync.dma_start(
                    out=xst_sb[:, 2 * ch:2 * ch + 2, :, :],
                    in_=xsi_r[:, 2 * ch:2 * ch + 2, :, :],
                )

            w2p_sb = constp.tile([128, 2 * L], fp16)
            nc.scalar.dma_start(out=w2p_sb[:], in_=w2p_d[:])
            b1p_sb = constp.tile([128, 2], f32)
            nc.scalar.dma_start(out=b1p_sb[:], in_=b1p_d[:])
            b2p_sb = constp.tile([L, 1], f32)
            nc.scalar.dma_start(out=b2p_sb[:], in_=b2p_d[:])
            # needed only by the s_r builds (~25us in): keep behind xst/wc
            iota_sb = constp.tile([128, SPAN_TILE], i8)
            nc.scalar.dma_start(out=iota_sb[:], in_=iota_d[:])
            i0v_sb = constp.tile([128, m_pad], i8)
            nc.scalar.dma_start(out=i0v_sb[:], in_=i0v_d[:])

            # unified A|G table: [128 parts, rank, 2 halves, 256]
            tab = tabp.tile([128, N_TT, 2, G_OFF], fp16)
            # zero the junk tails so gathered payload is NaN-free
            nc.vector.memset(tab[:, :, 0, H:G_OFF], 0.0)
            nc.vector.memset(tab[:, :, 1, H:G_OFF], 0.0)

            tabg_t = None if SBUF_TAB else dramp.tile([TC, G_OFF], fp16)

            def emit_gather(out_ap, idx_ap, n_idx, queue, sem, prep):
                src = tab[:] if SBUF_TAB else tabg_t[:, :]
                kw = dict(
                    sbuf_tokens_per_rank=128,
                    sbuf_free_dim_per_rank=2 * G_OFF * 2,
                    sbuf_byte_offset=G_OFF * 2,
                ) if SBUF_TAB else {}
                return nc.gpsimd.dma_gather(
                    out_ap, src, idx_ap, n_idx, n_idx,
                    elem_size=G_OFF, transpose=True, prepare_only=prep,
                    sem=sem, queue_num=queue, **kw,
                )

            gg_tiles = []
            gsems = []
            dummy_sems = []
            if PREP:
                # Tile does not defer the gather's table dep to the trigger,
                # so emit the preps BEFORE the table is written: desc-gen
                # reads only idx; the data read happens at trigger time, and
                # the triggers below carry explicit table deps.
                for st in range(n_st):
                    gg = ggp.tile([128, 2, SPAN_TILE], fp16, tag=f"gg_{st}",
                                  name=f"gg_{st}")
                    gg_tiles.append(gg)
                    gsem = nc.alloc_semaphore(f"gsem{st}")
                    gsems.append(gsem)
                    emit_gather(gg[:], idx_sb[:, st * 32:(st + 1) * 32],
                                SPAN_TILE, st % 4, gsem, True)
                for q in range(4):
                    dscr = ggp.tile([128, 2, 128], fp16, tag=f"gg_dummy{q}",
                                    name=f"gg_dummy{q}")
                    dsem = nc.alloc_semaphore(f"gsemd{q}")
                    dummy_sems.append(dsem)
                    emit_gather(dscr[:], idx_sb[:, 0:8], 128, q, dsem, True)

            # ---- A-side one-hot tiles (DVE 2x; overlaps stage-1 PE work) ----
            s_r = []
            for st in range(n_st):
                s0 = srp.tile([128, SPAN_TILE], fp16, tag=f"s0_{st}",
                              name=f"s0_{st}")
                nc.vector.tensor_tensor(
                    out=s0[:],
                    in0=iota_sb[:],
                    in1=i0v_sb[:, st * SPAN_TILE:(st + 1) * SPAN_TILE],
                    op=EQ,
                )
                s_r.append(s0)

            # ---- stage 1: per token tile, 8 K-block matmuls + one
            # PSUM->SBUF copy into the table stripe ----
            tab_dma = None
            tab_dmas = []
            for tt in range(N_TT):
                ps = ps1p.tile([128, W1N], f32)
                for kb in range(N_KB):
                    nc.tensor.matmul(
                        ps[:],
                        xst_sb[:, tt, kb, :],
                        wc_sb[:, kb * W1N:(kb + 1) * W1N],
                        start=(kb == 0),
                        stop=(kb == N_KB - 1),
                    )
                psv = ps[:].rearrange("p (h x) -> p h x", h=2)
                nc.scalar.activation(tab[:, tt, :, 0:H], psv, AF.Copy)
                if not SBUF_TAB:
                    # scalar ring: don't queue behind the big xst loads
                    tab_dma = nc.scalar.dma_start(
                        out=tabg_t[tt * 128:(tt + 1) * 128, :],
                        in_=tab[:, tt, 1, :],
                    )
                    tab_dmas.append(tab_dma)

            # ---- fire the prepared gathers (PREP) or emit direct gathers ----
            triggers = None
            if PREP:
                triggers = [
                    nc.gpsimd.trigger_dma(count=None, queue_num=q)
                    for q in range(4)
                ]
                # all 16 table writes (not mutually FIFO across queue rows):
                # pin every one on the first trigger; the rest follow it on
                # the GpSimd stream.
                deps = tab_dmas if not SBUF_TAB else []
                for d in deps:
                    add_dep_helper(triggers[0].ins, d.ins, True,
                                   "trigger after G table")
            else:
                for st in range(n_st):
                    gg = ggp.tile([128, 2, SPAN_TILE], fp16, tag=f"gg_{st}",
                                  name=f"gg_{st}")
                    gg_tiles.append(gg)
                    gsem = nc.alloc_semaphore(f"gsem{st}")
                    gsems.append(gsem)
                    g_inst = emit_gather(
                        gg[:], idx_sb[:, st * 32:(st + 1) * 32],
                        SPAN_TILE, st % 4, None, False,
                    ).then_inc(gsem, 16)
                    if not SBUF_TAB:
                        # first gather carries deps on ALL table writes (they
                        # are not mutually FIFO across queue rows); the rest
                        # follow on the GpSimd stream but keep the last-write
                        # dep for scheduler ordering.
                        deps = tab_dmas if st == 0 else [tab_dma]
                        for d in deps:
                            add_dep_helper(g_inst.ins, d.ins, True,
                                           "gather after G table")
                for q in range(4):
                    dscr = ggp.tile([128, 2, 128], fp16, tag=f"gg_dummy{q}",
                                    name=f"gg_dummy{q}")
                    dsem = nc.alloc_semaphore(f"gsemd{q}")
                    dummy_sems.append(dsem)
                    emit_gather(dscr[:], idx_sb[:, 0:8], 128, q, None,
                                False).then_inc(dsem, 16)

            # ---- stage 2 ----
            for st in range(n_st):
                gg = gg_tiles[st]
                s0 = s_r[st]
                pA0 = pA0p.tile([128, SPAN_TILE], f32, tag="pA0")
                pA1 = pA1p.tile([22, SPAN_TILE], f32, tag="pA1")
                for (q, lo, hi) in runs[st]:
                    nc.tensor.matmul(
                        pA0[:, lo:hi], tab[:, q, 0, 0:128], s0[:, lo:hi],
                        start=True, stop=True,
                    )
                    nc.tensor.matmul(
                        pA1[:, lo:hi], tab[:, q, 0, 128:H], s0[:, lo:hi],
                        start=True, stop=True,
                    )
                # pre-activation = A_onehot + b1 + G_gather, relu
                t0 = actp.tile([128, SPAN_TILE], fp16, tag="t0")
                t1 = actp.tile([22, SPAN_TILE], fp16, tag="t1")
                h0 = actp.tile([128, SPAN_TILE], fp16, tag="h0")
                h1 = actp.tile([22, SPAN_TILE], fp16, tag="h1")
                stt0 = nc.vector.scalar_tensor_tensor(
                    t0[:], pA0[:], b1p_sb[:, 0:1], gg[:, 0, :], ADD, ADD
                )
                stt1 = nc.vector.scalar_tensor_tensor(
                    t1[:], pA1[:], b1p_sb[0:22, 1:2], gg[0:22, 1, :], ADD, ADD
                )
                if PREP:
                    add_dep_helper(stt0.ins, triggers[st % 4].ins, True, "gg after trigger")
                    add_dep_helper(stt1.ins, triggers[st % 4].ins, True, "gg after trigger")
                # gg data only lands once the transfer completes AND the next
                # same-queue gather has pushed the xbar past it (drain
                # guarantee for the transpose-spray writes).
                stt0._wait_ge(gsems[st], 16)
                stt1._wait_ge(gsems[st], 16)
                nxt = st + 4
                guard = gsems[nxt] if nxt < n_st else dummy_sems[st % 4]
                gw = nc.vector.wait_ge(guard, 16)
                add_dep_helper(stt0.ins, gw.ins, True, "xbar drain guard")
                add_dep_helper(stt1.ins, gw.ins, True, "xbar drain guard")
                nc.scalar.activation(h0[:], t0[:], AF.Relu)
                nc.scalar.activation(h1[:], t1[:], AF.Relu)
                # W2 contraction
                ps2 = ps2p.tile([L, SPAN_TILE], f32, tag="ps2")
                nc.tensor.matmul(
                    ps2[:], w2p_sb[:, 0:L], h0[:], start=True, stop=False
                )
                nc.tensor.matmul(
                    ps2[:], w2p_sb[0:22, L:2 * L], h1[:], start=False, stop=True
                )
                ot = otp.tile([L, SPAN_TILE], f32)
                nc.scalar.activation(ot[:], ps2[:], AF.Identity, bias=b2p_sb[:])
                nc.sync.dma_start(
                    out=outT_d[:, st * SPAN_TILE:(st + 1) * SPAN_TILE], in_=ot[:]
                )

    return nc


def prep_inputs(xs, spans, batch_ids, W1, b1, W2, b2):
    """Host-side routing and layout.

    Returns (in_maps, scatter info, m_pad, runs)."""
    xs = np.asarray(xs, dtype=np.float32)
    spans = np.asarray(spans).astype(np.int64)
    batch_ids = np.asarray(batch_ids).astype(np.int64)
    W1 = np.asarray(W1, dtype=np.float32)
    b1 = np.asarray(b1, dtype=np.float32)
    W2 = np.asarray(W2, dtype=np.float32)
    b2 = np.asarray(b2, dtype=np.float32)

    core = batch_ids // BPC
    local0 = (batch_ids % BPC) * T + spans[:, 0]
    local1 = (batch_ids % BPC) * T + spans[:, 1]

    # bucket spans by (core, i0 block); slot template = per-block max
    q0 = local0 >> 7
    counts = np.zeros((NCORES, N_TT), np.int64)
    np.add.at(counts, (core, q0), 1)
    C = counts.max(axis=0)                      # [16] block capacities
    total = int(C.sum())
    m_pad = int(max(np.ceil(total / SPAN_TILE), 1) * SPAN_TILE)
    C[N_TT - 1] += m_pad - total                # absorb tail padding
    S = np.concatenate([[0], np.cumsum(C)])     # block slot offsets
    n_st = m_pad // SPAN_TILE

    # compile-time column runs per span tile
    runs = []
    for st in range(n_st):
        t_lo, t_hi = st * SPAN_TILE, (st + 1) * SPAN_TILE
        rl = []
        for q in range(N_TT):
            lo = max(t_lo, int(S[q]))
            hi = min(t_hi, int(S[q + 1]))
            if lo < hi:
                rl.append((q, lo - t_lo, hi - t_lo))
        runs.append(rl)

    # shared weights
    W1h = W1.astype(np.float16)
    wc = np.empty((128, N_KB * W1N), np.float16)
    for kb in range(N_KB):
        wc[:, kb * W1N:kb * W1N + H] = W1h[kb * 128:(kb + 1) * 128, :]
        wc[:, kb * W1N + H:(kb + 1) * W1N] = W1h[D + kb * 128:D + (kb + 1) * 128, :]
    W2pad = np.zeros((2 * 128, L), np.float16)
    W2pad[:H] = W2.astype(np.float16)
    w2p = np.empty((128, 2 * L), np.float16)
    w2p[:, 0:L] = W2pad[0:128]
    w2p[:, L:2 * L] = W2pad[128:256]
    b1pad = np.zeros((256,), np.float32)
    b1pad[:H] = b1
    b1p = np.ascontiguousarray(b1pad.reshape(2, 128).T)
    b2p = np.ascontiguousarray(b2.reshape(L, 1))

    iota = np.ascontiguousarray(
        np.broadcast_to(np.arange(128, dtype=np.int8)[:, None],
                        (128, SPAN_TILE))
    )

    # per-core slot assignment into the shared template
    scatter = []                                 # (orig span ids, slot ids)
    in_maps = []
    for c in range(NCORES):
        sel = np.nonzero(core == c)[0]
        qs = q0[sel]
        order = np.argsort(qs, kind="stable")
        sel = sel[order]
        qs = qs[order]
        # slots: for block q, spans land at S[q] + [0, n_cq)
        slot = np.empty(len(sel), np.int64)
        pos = 0
        i0 = np.zeros(m_pad, np.int64)
        i1 = np.zeros(m_pad, np.int64)
        # dummy defaults: block-consistent begin token, end token 0
        for q in range(N_TT):
            i0[int(S[q]):int(S[q + 1])] = q * 128
        for q in range(N_TT):
            n_cq = int(counts[c, q])
            slot[pos:pos + n_cq] = int(S[q]) + np.arange(n_cq)
            pos += n_cq
        i0[slot] = local0[sel]
        i1[slot] = local1[sel]
        scatter.append((sel, slot))

        # G gather indices: position i -> partition i%16, col i//16, 8x rep
        arr16 = i1.reshape(m_pad // 16, 16).T
        idxc = np.ascontiguousarray(np.tile(arr16, (8, 1)).astype(np.int16))
        # A-side one-hot compare values: r = i0 % 128, replicated
        i0r = (i0 & 127).astype(np.int8)
        i0v = np.ascontiguousarray(np.broadcast_to(i0r, (128, m_pad)))

        xs_c = xs[c * BPC:(c + 1) * BPC].reshape(TC, D)
        # interleave: [p, tt, kb, j] = xs_c[tt*128 + j, kb*128 + p]
        xsi = np.ascontiguousarray(
            xs_c.astype(np.float16)
            .reshape(N_TT, 128, N_KB, 128)
            .transpose(3, 0, 2, 1)
            .reshape(128, N_TT * N_KB * 128)
        )
        in_maps.append({
            "xsi": xsi, "wc": wc, "w2p": w2p, "b1p": b1p, "b2p": b2p,
            "idx": idxc, "i0v": i0v, "iota": iota,
        })

    return in_maps, scatter, m_pad, runs


def _scatter_out(results, scatter):
    out = np.empty((N_SPANS, L), np.float32)
    for c in range(NCORES):
        sel, slot = scatter[c]
        out[sel] = results[c]["outT"].T[slot]
    return out


def _install_ntff_shim():
    """Provide antenv.axon_hooks (missing on this image) so that
    run_bass_kernel_spmd(trace=True) can drive NTFF profiling via the
    axon .so. Only used by the profiling path."""
    import sys
    import types
    import ctypes
    import contextlib

    if "antenv.axon_hooks" in sys.modules:
        return
    import antenv

    holder = {"hook": None}
    mod = types.ModuleType("antenv.axon_hooks")
    mod.set_axon_ntff_profile_hook = lambda h: holder.__setitem__("hook", h)
    mod.get_axon_ntff_profile_hook = lambda: holder["hook"]
    sys.modules["antenv.axon_hooks"] = mod
    antenv.axon_hooks = mod

    so_path = "/opt/axon/libaxon_pjrt.so"
    try:
        lib = ctypes.CDLL(so_path)
    except OSError:
        return
    if not hasattr(lib, "axon_start_nrt_profile"):
        return
    lib.axon_start_nrt_profile.argtypes = [
        ctypes.POINTER(ctypes.c_int64),
        ctypes.c_size_t,
    ]
    lib.axon_start_nrt_profile.restype = ctypes.c_int64
    lib.axon_stop_nrt_profile.argtypes = [ctypes.c_char_p]
    lib.axon_stop_nrt_profile.restype = ctypes.c_int64

    @contextlib.contextmanager
    def _hook(output_dir, device_ids):
        import jax

        jax.devices()
        if device_ids:
            ids = (ctypes.c_int64 * len(device_ids))(*device_ids)
            rc = lib.axon_start_nrt_profile(ids, len(device_ids))
        else:
            rc = lib.axon_start_nrt_profile(None, 0)
        if rc != 0:
            raise RuntimeError(f"axon_start_nrt_profile rc={rc}")
        try:
            yield
        finally:
            n = lib.axon_stop_nrt_profile(str(output_dir).encode())
            print(f"profile: {n} file(s) written to {output_dir}")

    mod.set_axon_ntff_profile_hook(_hook)


def run(inputs: dict, trace: bool = False):
    """Run on the 8 NeuronCores. Returns (out, BassKernelResults)."""
    from concourse import bass_utils
    from concourse.bass_utils import run_bass_kernel_spmd

    if trace:
        _install_ntff_shim()
        bass_utils.upload_artifacts = lambda tmpdir: str(tmpdir)

    in_maps, scatter, m_pad, runs = prep_inputs(**inputs)
    nc = build_graph(m_pad, runs)
    nc.finalize()
    res = run_bass_kernel_spmd(
        nc, in_maps, list(range(NCORES)), trace=trace
    )
    return _scatter_out(res.results, scatter), res


def kernel(**inputs) -> np.ndarray:
    out, _ = run(inputs, trace=False)
    return out
